# revision 13
# baseline (speedup 1.0000x reference)
"""AIMv2 attention (B=4, S=2048, D=1024, H=16, d=64) on 8 TRN2 NeuronCores.

Sharding: core c = (batch b = c//2, head-group g = c%2 of 8 heads).
Each core computes its batch's attention for its 8 heads plus the
out-projection partial sum over its heads' rows of w_out; the host adds
the two partials per batch (no on-device collectives needed).

Per-core kernel (all matmuls in bf16, fp32 accumulation; inputs are
pre-cast to bf16 on the host so no on-chip casts or fp32 staging):
  X^T via TensorE 128x128 transposes (keeps the PE HAM warm through the
  DMA phase); Q^T,K^T = Wq/k^T @ X^T so the score matmuls produce
  s_T[k, q] directly; softmax without max-subtraction (scores ~ N(0,1),
  exp never overflows fp32/bf16); V carries a ones column so
  ctx' = [V|1]^T @ P^T yields both ctx^T and the softmax denominators in
  one PSUM accumulation; normalization uses a K=1 broadcast matmul +
  reciprocal_approx_fast (exact reciprocal is 5x slower, and the approx
  op misbehaves at base_partition 64, hence the broadcast-first order).

  The attention loop processes ONE head x one 1024-q chunk per attend
  (16 attends of 16 k-tiles).  A single-head score tile is [128,1024]f32
  = 2 PSUM banks, so the score pipeline gets bufs=3 (6 banks) and the
  single ctx' accumulator [65,1024]f32 the remaining 2 banks.  The
  3-deep score rotation decouples the PE from ScalarE's exp stream (a
  2-deep rotation locksteps the two engines at the exp period and every
  injected matmul stretches the loop).  All remaining work - Q/K
  projection bursts for head-groups 1-3 and the first half of the
  out-projection - is a gated work queue drained one burst per few
  k-tiles inside the loop, sized to the PE's per-k-tile slack vs exp;
  cross-attend cleanup (last LAG ctx tiles, PSUM evacuation,
  normalization) drains during the first k-tiles of the next attend.
"""

import ml_dtypes
import numpy as np

import concourse.tile as tile
from concourse import bacc, mybir
from concourse.bass_utils import run_bass_kernel_spmd
from concourse.masks import make_identity

P = 128
S = 2048          # sequence length
D = 1024          # model dim
DQ = 512          # per-core qkv width (8 heads x 64)
HD = 64           # head dim
NH = 8            # heads per core
NKT = D // P      # 8 contraction tiles over D
NST = S // P      # 16 tiles over S
QC = 1024         # q chunk for attention inner loop
LAG = 6           # ctx matmul lag behind scores/exp in the pipeline
SCALE = 1.0 / 8.0  # 1/sqrt(64)

F32 = mybir.dt.float32
BF16 = mybir.dt.bfloat16


def build_kernel(nc, out_ap, hs_ap, wqkv_ap, wout_ap):
    import contextlib

    ctx = contextlib.ExitStack()
    with tile.TileContext(nc) as tc:
        with ctx:
            _body(ctx, tc, nc, out_ap, hs_ap, wqkv_ap, wout_ap)


def _body(ctx, tc, nc, out_ap, hs_ap, wqkv_ap, wout_ap):
    Exp = mybir.ActivationFunctionType.Exp

    persist = ctx.enter_context(tc.tile_pool(name="persist", bufs=1))
    psum = ctx.enter_context(tc.tile_pool(name="psum", bufs=1, space="PSUM"))

    # all-ones [128, 64] so a ones-row lhsT can be sliced at any base
    # partition (matmul requires lhsT/rhs base partitions to match)
    ones_rows = persist.tile([P, HD], BF16, name="ones_rows")
    nc.vector.memset(ones_rows[:], 1.0)

    wout_bf = []
    vc = [persist.tile([P, NH, HD + 1], BF16, name=f"vc{st}") for st in range(NST)]
    qt = [persist.tile([P, S], BF16, name=f"qt{m}") for m in range(4)]
    kt_sb = [persist.tile([P, S], BF16, name=f"kt{m}") for m in range(4)]
    ctxt = [persist.tile([P, S], BF16, name=f"ctxt{m}") for m in range(4)]

    pt_pool = ctx.enter_context(tc.tile_pool(name="pt", bufs=12))
    small = ctx.enter_context(tc.tile_pool(name="small", bufs=4))
    outsb_pool = ctx.enter_context(tc.tile_pool(name="outsb", bufs=3))
    pstage = ctx.enter_context(tc.tile_pool(name="pstage", bufs=3))

    # ================= head: loads + all projections =================
    # Inputs arrive pre-cast to bf16 (host-side), so weights DMA straight
    # into their bf16 tiles; X^T is built by TensorE 128x128 transposes.
    # proj_scope (X^T + qkv weights) is released once the last queued
    # projection burst has been emitted, mid-attention.
    pscope = tc.alloc_tile_pool(name="proj_scope", bufs=1)
    if True:
        # sts 0-11 transpose on TensorE into xt3a; sts 12-15 go through the
        # XBAR DMA path (single engine - two-engine xbar use corrupts) into a
        # SEPARATE tensor so the two paths share no WAW/WAR dependences.
        xt3a = pscope.tile([P, NKT, 12 * P], BF16, name="xt3a")
        xt3b = pscope.tile([P, NKT, 4 * P], BF16, name="xt3b")
        wqkv_bf = []

        identity = pscope.tile([P, P], BF16, name="identity")
        make_identity(nc, identity[:])

        # free-running warm-up burst: ~4us of tiny matmuls flips the PE HAM
        # to K=8/8 before the transpose/projection phase so the (PE-bound)
        # head doesn't run at 1.2 GHz when the kernel lands on a cold HAM
        # window; kept alive through DCE via the 0-scaled add below
        warm_ps = psum.tile([HD, HD], F32, tag="ctx", bufs=1, name="warm_ps")
        N_WARM = 72
        for wi in range(N_WARM):
            nc.tensor.matmul(
                warm_ps[:], lhsT=ones_rows[0:HD, :], rhs=ones_rows[0:HD, :],
                start=(wi == 0), stop=(wi == N_WARM - 1),
            )
        warmsb = pscope.tile([HD, HD], F32, name="warmsb")
        nc.vector.tensor_scalar_mul(warmsb[:], warm_ps[:], 0.0)
        nc.vector.tensor_add(ones_rows[0:HD, :], ones_rows[0:HD, :], warmsb[:])

        # weight DMAs ride the sync queue; the X row-tiles (which gate the
        # PE's transpose work) go first on the scalar queue, wout last
        for kt in range(NKT):
            wb = pscope.tile([P, 3 * DQ], BF16, name=f"wqkv_bf{kt}")
            nc.sync.dma_start(wb[:], wqkv_ap[kt * P:(kt + 1) * P, :])
            wqkv_bf.append(wb)

        # X: load bf16 row-tiles, transpose 128x128 blocks on TensorE (PE is
        # otherwise idle here and this keeps HAM warm), evacuate per-row-tile
        with tc.tile_pool(name="stage", bufs=4) as stage:
            for st in range(NST):
                xb = stage.tile([P, D], BF16, tag="xbf", bufs=4)
                nc.scalar.dma_start(xb[:], hs_ap[st * P:(st + 1) * P, :])
                if st < 12:
                    ps_t = psum.tile([P, D], BF16, tag="sc", bufs=3, name="ps_t")
                    for dt in range(NKT):
                        nc.tensor.transpose(
                            ps_t[:, dt * P:(dt + 1) * P],
                            xb[:, dt * P:(dt + 1) * P],
                            identity[:],
                        )
                    nc.vector.tensor_copy(
                        xt3a[:, :, st * P:(st + 1) * P],
                        ps_t[:].rearrange("p (h e) -> p h e", h=NKT),
                    )
                else:
                    nc.sync.dma_start_transpose(
                        xt3b[:, :, (st - 12) * P:(st - 11) * P], xb[:]
                    )

        for i in range(DQ // P):
            wb = persist.tile([P, D], BF16, name=f"wout_bf{i}")
            nc.scalar.dma_start(wb[:], wout_ap[i * P:(i + 1) * P, :])
            wout_bf.append(wb)

        def xt_sl(kt, lo, width):
            # column slice [lo, lo+width) of X^T row-block kt
            if lo + width <= 12 * P:
                return xt3a[:, kt, lo:lo + width]
            assert lo >= 12 * P
            return xt3b[:, kt, lo - 12 * P:lo - 12 * P + width]

        # V projection with ones column: vc[st][:, h, 0:64]=V_h, [...,64]=1
        for st in range(NST):
            nc.vector.memset(vc[st][:, :, HD:HD + 1], 1.0)
        for stq in range(NST // 2):
            ps = psum.tile([P, 2 * DQ], F32, tag="sc", bufs=3)
            for half in range(2):
                st = 2 * stq + half
                sl = slice(half * DQ, (half + 1) * DQ)
                for kt in range(NKT):
                    nc.tensor.matmul(
                        ps[:, sl],
                        lhsT=xt_sl(kt, st * P, P),
                        rhs=wqkv_bf[kt][:, 2 * DQ:3 * DQ],
                        start=(kt == 0),
                        stop=(kt == NKT - 1),
                    )
            for half in range(2):
                st = 2 * stq + half
                src = ps[:, half * DQ:(half + 1) * DQ].rearrange(
                    "p (h e) -> p h e", h=NH
                )
                nc.vector.tensor_copy(vc[st][:, :, 0:HD], src)

        # Q^T / K^T projection bursts: one [128, 512] chunk = 8 matmuls
        # + 1 copy (~1.7us). Head-group 0's K (full S) + first q-chunk of Q
        # plus head-group 1's leading chunks are emitted in the head; the
        # rest drain inside the attention loop as 4-matmul half-bursts.
        def proj_burst(m, which, nq, width=512):
            dst = qt[m] if which == 0 else kt_sb[m]
            ps = psum.tile([P, width], F32, tag="sc", bufs=3, name="projps")
            for kt in range(NKT):
                nc.tensor.matmul(
                    ps[:],
                    lhsT=wqkv_bf[kt][:, which + m * P: which + (m + 1) * P],
                    rhs=xt_sl(kt, nq * width, width),
                    start=(kt == 0),
                    stop=(kt == NKT - 1),
                )
            nc.vector.tensor_copy(dst[:, nq * width:(nq + 1) * width], ps[:])

        for nq in range(4):
            proj_burst(0, DQ, nq)        # K^T head-group 0, full S
        proj_burst(0, 0, 0)              # Q^T head-group 0, q-chunk 0
        proj_burst(0, 0, 1)
        proj_burst(1, DQ, 0)             # head-group 1 leading chunks
        proj_burst(1, 0, 0)
        proj_burst(1, 0, 1)

    released = [False]

    def release_scope():
        if not released[0]:
            pscope.release()
            released[0] = True

    # In-loop injection is quarter-bursts: 2 accumulating matmuls (~0.4us
    # = one k-tile's PE slack vs the exp period) into one PSUM bank,
    # chained across quarters via an f32 SBUF stash on VectorE. Larger
    # units exceed the score-rotation's elasticity (ScalarE has at most
    # bufs-1 score tiles buffered) and stall the exp stream; 8-matmul
    # bursts demonstrably stall it by their full duration.
    stash = {}

    def proj_q(m, which, nq, kq):
        dst = qt[m] if which == 0 else kt_sb[m]
        key = (m, which, nq)
        ps = psum.tile([P, 512], F32, tag="sc", bufs=3, name="projps")
        for kt in range(2 * kq, 2 * kq + 2):
            nc.tensor.matmul(
                ps[:],
                lhsT=wqkv_bf[kt][:, which + m * P: which + (m + 1) * P],
                rhs=xt_sl(kt, nq * 512, 512),
                start=(kt == 2 * kq),
                stop=(kt == 2 * kq + 1),
            )
        if kq == 0:
            stg = pstage.tile([P, 512], F32, tag="pstg", bufs=3)
            nc.vector.tensor_copy(stg[:], ps[:])
            stash[key] = stg
        elif kq < 3:
            stg = stash[key]
            nc.vector.tensor_add(stg[:], ps[:], stg[:])
        else:
            nc.vector.tensor_add(
                dst[:, nq * 512:(nq + 1) * 512], ps[:], stash.pop(key)
            )

    # gated in-loop work queue: (gate_attend_idx, closure). Items pop in
    # order; a gated head blocks the slot (ordering preserves readiness).
    # Per head-group m: K n0 then Q qc0 halves then K n1-3 - the consuming
    # attend reads K column-blocks progressively (subtile deps) but needs
    # both Q halves of its q-chunk from k-tile 0.
    work_q = []

    def q_burst(gate, m, which, nq):
        for kq in range(4):
            work_q.append(
                (gate, lambda m=m, w=which, nq=nq, kq=kq: proj_q(m, w, nq, kq))
            )

    for nq in range(1, 4):
        q_burst(0, 1, DQ, nq)            # rest of K^T head-group 1
    for m in range(2, 4):
        q_burst(0, m, DQ, 0)
        q_burst(0, m, 0, 0)
        q_burst(0, m, 0, 1)
        for nq in range(1, 4):
            q_burst(0, m, DQ, nq)
    for m in range(4):
        q_burst(0, m, 0, 2)              # q-chunk-1 Q halves
        q_burst(0, m, 0, 3)
    work_q.append((0, lambda: release_scope()))

    def outproj_q(st, half, cpair):
        sl = slice(half * 512, (half + 1) * 512)
        ps = psum.tile([P, 512], F32, tag="sc", bufs=3, name="ops")
        for c in (2 * cpair, 2 * cpair + 1):
            nc.tensor.matmul(
                ps[:],
                lhsT=ctxt[c][:, st * P:(st + 1) * P],
                rhs=wout_bf[c][:, sl],
                start=(c == 2 * cpair),
                stop=(c == 2 * cpair + 1),
            )
        if (half, cpair) == (0, 0):
            stash[("o", st)] = outsb_pool.tile(
                [P, D], F32, tag="osb", bufs=3, name="osb"
            )
        osb = stash[("o", st)]
        if cpair == 0:
            stg = pstage.tile([P, 512], F32, tag="pstg", bufs=3)
            nc.vector.tensor_copy(stg[:], ps[:])
            stash[("og", st, half)] = stg
        else:
            nc.vector.tensor_add(osb[:, sl], ps[:], stash.pop(("og", st, half)))
        if (half, cpair) == (1, 1):
            stash.pop(("o", st))
            eng = (nc.sync, nc.scalar)[st % 2]
            eng.dma_start(out_ap[st * P:(st + 1) * P, :], osb[:])

    def outproj(st):
        for half in range(2):
            for cpair in range(2):
                outproj_q(st, half, cpair)

    # out-projection first half: q-chunk-0 rows, ready once every head's
    # qc=0 normalization has drained (during attend 8's first k-tiles)
    for st in range(NST // 2):
        for half in range(2):
            for cpair in range(2):
                work_q.append(
                    (8, lambda st=st, h=half, c=cpair: outproj_q(st, h, c))
                )

    # ================= attention =================
    # deferred cross-attend work: closures drained 2-per-k-tile during the
    # first LAG k-tiles of the following attend (which has no ctx work)
    pending = []

    def drain(n):
        for _ in range(min(n, len(pending))):
            pending.pop(0)()

    def normalize(csb, m, r0, qc):
        """ctx^T[d,q] /= sum[q] (sums in row 64 of csb)."""
        q0 = qc * QC
        bc = psum.tile([HD, QC], F32, tag="sc", bufs=3)
        for half in range(2):
            sl = slice(half * 512, (half + 1) * 512)
            nc.tensor.matmul(
                bc[:, sl], lhsT=ones_rows[HD:HD + 1, :],
                rhs=csb[HD:HD + 1, sl],
                start=True, stop=True,
            )
        rec = small.tile([HD, QC], F32, tag="rec", bufs=2)
        nc.vector.reciprocal_approx_fast(rec[:], bc[:])
        nc.vector.tensor_mul(
            ctxt[m][r0:r0 + HD, q0:q0 + QC], csb[0:HD, :], rec[:]
        )

    def attend(aidx, h, qc):
        """One head x one 1024-q chunk; even head on partitions 0-63 of
        qt/kt_sb[m], odd head on 64-127."""
        q0 = qc * QC
        m, r0 = h // 2, HD * (h % 2)
        state = {}

        def emit_scores(kti):
            ps = psum.tile([P, QC], F32, tag="sc", bufs=3)
            for half in range(2):
                sl = slice(half * 512, (half + 1) * 512)
                qsl = slice(q0 + half * 512, q0 + (half + 1) * 512)
                nc.tensor.matmul(
                    ps[:, sl],
                    lhsT=kt_sb[m][r0:r0 + HD, kti * P:(kti + 1) * P],
                    rhs=qt[m][r0:r0 + HD, qsl],
                    start=True, stop=True,
                )
            return ps

        def emit_ctx(kti, pt):
            if kti == 0:
                state["ctx"] = psum.tile(
                    [HD + 1, QC], F32, tag="ctx", bufs=1, name="ctxp"
                )
            first = kti == 0
            last = kti == NST - 1
            for half in range(2):
                sl = slice(half * 512, (half + 1) * 512)
                nc.tensor.matmul(
                    state["ctx"][:, sl], lhsT=vc[kti][:, h, :],
                    rhs=pt[:, sl], start=first, stop=last,
                )

        pts = {}
        if aidx == 0:
            pop_kts = set(range(1, NST))
        elif aidx <= 5:
            pop_kts = {4, 6, 8, 9, 10, 11, 12, 14, 15}
        else:
            pop_kts = {4, 6, 8, 10, 12, 14, 15}
        for kti in range(NST):
            ps = emit_scores(kti)
            if kti < LAG:
                drain(1)           # previous attend's tail ctx, 1 per k-tile
            else:
                emit_ctx(kti - LAG, pts.pop(kti - LAG))
                if kti < LAG + 2:
                    drain(1)       # evac (kt LAG) + normalize (kt LAG+1)
            if kti in pop_kts and work_q and work_q[0][0] <= aidx:
                work_q.pop(0)[1]()
            pt = pt_pool.tile([P, QC], BF16, tag="pt", bufs=12)
            nc.scalar.activation(pt[:], ps[:], Exp, scale=SCALE)
            pts[kti] = pt

        # tail: last LAG ctx tiles + PSUM evacuation + normalization are
        # deferred into the next attend's first k-tiles
        def tail_ctx(kti):
            def f():
                emit_ctx(kti, pts.pop(kti))
            return f

        for kti in range(NST - LAG, NST):
            pending.append(tail_ctx(kti))

        def evac():
            csb = small.tile([HD + 1, QC], BF16, tag="csb", bufs=4)
            nc.vector.tensor_copy(csb[:], state["ctx"][:])
            state["csb"] = csb

        pending.append(evac)
        pending.append(lambda: normalize(state["csb"], m, r0, qc))

    aidx = 0
    for qc in range(2):
        for h in range(NH):
            attend(aidx, h, qc)
            aidx += 1
    while work_q:
        work_q.pop(0)[1]()
    drain(len(pending))
    for st in range(NST // 2, NST):
        outproj(st)


_CACHED = None


def _get_nc():
    global _CACHED
    if _CACHED is None:
        nc = bacc.Bacc(
            "TRN2", target_bir_lowering=False, debug=False, num_devices=8
        )
        hs = nc.dram_tensor("hs", [S, D], BF16, kind="ExternalInput").ap()
        wqkv = nc.dram_tensor("wqkv", [D, 3 * DQ], BF16, kind="ExternalInput").ap()
        wout = nc.dram_tensor("wout", [DQ, D], BF16, kind="ExternalInput").ap()
        out = nc.dram_tensor("out", [S, D], F32, kind="ExternalOutput").ap()
        build_kernel(nc, out, hs, wqkv, wout)
        nc.compile()
        _CACHED = nc
    return _CACHED


def make_in_maps(hidden_states, w_qkv, w_out):
    in_maps = []
    for c in range(8):
        b, g = divmod(c, 2)
        cols = slice(g * DQ, (g + 1) * DQ)
        wq = w_qkv[:, 0 * D:1 * D][:, cols]
        wk = w_qkv[:, 1 * D:2 * D][:, cols]
        wv = w_qkv[:, 2 * D:3 * D][:, cols]
        bf = ml_dtypes.bfloat16
        in_maps.append({
            "hs": np.ascontiguousarray(hidden_states[b]).astype(bf),
            "wqkv": np.ascontiguousarray(
                np.concatenate([wq, wk, wv], axis=1)
            ).astype(bf),
            "wout": np.ascontiguousarray(
                w_out[g * DQ:(g + 1) * DQ, :]
            ).astype(bf),
        })
    return in_maps


def run(hidden_states, w_qkv, w_out, trace=False):
    nc = _get_nc()
    in_maps = make_in_maps(hidden_states, w_qkv, w_out)
    res = None
    last_err = None
    for _attempt in range(3):
        try:
            res = run_bass_kernel_spmd(
                nc, in_maps, core_ids=list(range(8)), trace=trace
            )
            break
        except Exception as e:  # transient NRT/device hiccups
            last_err = e
    if res is None:
        raise last_err
    out = np.empty((4, S, D), np.float32)
    for b in range(4):
        out[b] = res.results[2 * b]["out"] + res.results[2 * b + 1]["out"]
    return out, res


def kernel(hidden_states, w_qkv, w_out):
    out, _ = run(
        np.asarray(hidden_states), np.asarray(w_qkv), np.asarray(w_out)
    )
    return out


# revision 15
# speedup vs baseline: 1.0029x; 1.0029x over previous
"""AIMv2 attention (B=4, S=2048, D=1024, H=16, d=64) on 8 TRN2 NeuronCores.

Sharding: core c = (batch b = c//2, head-group g = c%2 of 8 heads).
Each core computes its batch's attention for its 8 heads plus the
out-projection partial sum over its heads' rows of w_out; the host adds
the two partials per batch (no on-device collectives needed).

Per-core kernel (all matmuls in bf16, fp32 accumulation; inputs are
pre-cast to bf16 on the host so no on-chip casts or fp32 staging):
  X^T via TensorE 128x128 transposes (keeps the PE HAM warm through the
  DMA phase); Q^T,K^T = Wq/k^T @ X^T so the score matmuls produce
  s_T[k, q] directly; softmax without max-subtraction (scores ~ N(0,1),
  exp never overflows fp32/bf16); V carries a ones column so
  ctx' = [V|1]^T @ P^T yields both ctx^T and the softmax denominators in
  one PSUM accumulation; normalization uses a K=1 broadcast matmul +
  reciprocal_approx_fast (exact reciprocal is 5x slower, and the approx
  op misbehaves at base_partition 64, hence the broadcast-first order).

  The attention loop processes ONE head x one 1024-q chunk per attend
  (16 attends of 16 k-tiles).  A single-head score tile is [128,1024]f32
  = 2 PSUM banks, so the score pipeline gets bufs=3 (6 banks) and the
  single ctx' accumulator [65,1024]f32 the remaining 2 banks.  The
  3-deep score rotation decouples the PE from ScalarE's exp stream (a
  2-deep rotation locksteps the two engines at the exp period and every
  injected matmul stretches the loop).  All remaining work - Q/K
  projection bursts for head-groups 1-3 and the first half of the
  out-projection - is a gated work queue drained one burst per few
  k-tiles inside the loop, sized to the PE's per-k-tile slack vs exp;
  cross-attend cleanup (last LAG ctx tiles, PSUM evacuation,
  normalization) drains during the first k-tiles of the next attend.
"""

import ml_dtypes
import numpy as np

import concourse.tile as tile
from concourse import bacc, mybir
from concourse.bass_utils import run_bass_kernel_spmd
from concourse.masks import make_identity

P = 128
S = 2048          # sequence length
D = 1024          # model dim
DQ = 512          # per-core qkv width (8 heads x 64)
HD = 64           # head dim
NH = 8            # heads per core
NKT = D // P      # 8 contraction tiles over D
NST = S // P      # 16 tiles over S
QC = 1024         # q chunk for attention inner loop
LAG = 6           # ctx matmul lag behind scores/exp in the pipeline
SCALE = 1.0 / 8.0  # 1/sqrt(64)

F32 = mybir.dt.float32
BF16 = mybir.dt.bfloat16


def build_kernel(nc, out_ap, hs_ap, wqkv_ap, wout_ap):
    import contextlib

    ctx = contextlib.ExitStack()
    with tile.TileContext(nc) as tc:
        with ctx:
            _body(ctx, tc, nc, out_ap, hs_ap, wqkv_ap, wout_ap)


def _body(ctx, tc, nc, out_ap, hs_ap, wqkv_ap, wout_ap):
    Exp = mybir.ActivationFunctionType.Exp

    persist = ctx.enter_context(tc.tile_pool(name="persist", bufs=1))
    psum = ctx.enter_context(tc.tile_pool(name="psum", bufs=1, space="PSUM"))

    # all-ones [128, 64] so a ones-row lhsT can be sliced at any base
    # partition (matmul requires lhsT/rhs base partitions to match)
    ones_rows = persist.tile([P, HD], BF16, name="ones_rows")
    nc.vector.memset(ones_rows[:], 1.0)

    wout_bf = []
    vc = [persist.tile([P, NH, HD + 1], BF16, name=f"vc{st}") for st in range(NST)]
    qt = [persist.tile([P, S], BF16, name=f"qt{m}") for m in range(4)]
    kt_sb = [persist.tile([P, S], BF16, name=f"kt{m}") for m in range(4)]
    ctxt = [persist.tile([P, S], BF16, name=f"ctxt{m}") for m in range(4)]

    pt_pool = ctx.enter_context(tc.tile_pool(name="pt", bufs=12))
    small = ctx.enter_context(tc.tile_pool(name="small", bufs=4))
    outsb_pool = ctx.enter_context(tc.tile_pool(name="outsb", bufs=3))
    pstage = ctx.enter_context(tc.tile_pool(name="pstage", bufs=3))

    # ================= head: loads + all projections =================
    # Inputs arrive pre-cast to bf16 (host-side), so weights DMA straight
    # into their bf16 tiles; X^T is built by TensorE 128x128 transposes.
    # proj_scope (X^T + qkv weights) is released once the last queued
    # projection burst has been emitted, mid-attention.
    pscope = tc.alloc_tile_pool(name="proj_scope", bufs=1)
    if True:
        # sts 0-11 transpose on TensorE into xt3a; sts 12-15 go through the
        # XBAR DMA path (single engine - two-engine xbar use corrupts) into a
        # SEPARATE tensor so the two paths share no WAW/WAR dependences.
        xt3a = pscope.tile([P, NKT, 12 * P], BF16, name="xt3a")
        xt3b = pscope.tile([P, NKT, 4 * P], BF16, name="xt3b")
        wqkv_bf = []

        identity = pscope.tile([P, P], BF16, name="identity")
        make_identity(nc, identity[:])

        # free-running warm-up burst: ~4us of tiny matmuls flips the PE HAM
        # to K=8/8 before the transpose/projection phase so the (PE-bound)
        # head doesn't run at 1.2 GHz when the kernel lands on a cold HAM
        # window; kept alive through DCE via the 0-scaled add below
        warm_ps = psum.tile([HD, HD], F32, tag="ctx", bufs=1, name="warm_ps")
        N_WARM = 48
        for wi in range(N_WARM):
            nc.tensor.matmul(
                warm_ps[:], lhsT=ones_rows[0:HD, :], rhs=ones_rows[0:HD, :],
                start=(wi == 0), stop=(wi == N_WARM - 1),
            )
        warmsb = pscope.tile([HD, HD], F32, name="warmsb")
        nc.vector.tensor_scalar_mul(warmsb[:], warm_ps[:], 0.0)
        nc.vector.tensor_add(ones_rows[0:HD, :], ones_rows[0:HD, :], warmsb[:])

        # weight DMAs ride the sync queue; the X row-tiles (which gate the
        # PE's transpose work) go first on the scalar queue, wout last
        for kt in range(NKT):
            wb = pscope.tile([P, 3 * DQ], BF16, name=f"wqkv_bf{kt}")
            nc.sync.dma_start(wb[:], wqkv_ap[kt * P:(kt + 1) * P, :])
            wqkv_bf.append(wb)

        # X: load bf16 row-tiles, transpose 128x128 blocks on TensorE (PE is
        # otherwise idle here and this keeps HAM warm), evacuate per-row-tile
        with tc.tile_pool(name="stage", bufs=4) as stage:
            for st in range(NST):
                xb = stage.tile([P, D], BF16, tag="xbf", bufs=4)
                nc.scalar.dma_start(xb[:], hs_ap[st * P:(st + 1) * P, :])
                if st < 12:
                    ps_t = psum.tile([P, D], BF16, tag="sc", bufs=3, name="ps_t")
                    for dt in range(NKT):
                        nc.tensor.transpose(
                            ps_t[:, dt * P:(dt + 1) * P],
                            xb[:, dt * P:(dt + 1) * P],
                            identity[:],
                        )
                    nc.vector.tensor_copy(
                        xt3a[:, :, st * P:(st + 1) * P],
                        ps_t[:].rearrange("p (h e) -> p h e", h=NKT),
                    )
                else:
                    nc.sync.dma_start_transpose(
                        xt3b[:, :, (st - 12) * P:(st - 11) * P], xb[:]
                    )

        for i in range(DQ // P):
            wb = persist.tile([P, D], BF16, name=f"wout_bf{i}")
            nc.scalar.dma_start(wb[:], wout_ap[i * P:(i + 1) * P, :])
            wout_bf.append(wb)

        def xt_sl(kt, lo, width):
            # column slice [lo, lo+width) of X^T row-block kt
            if lo + width <= 12 * P:
                return xt3a[:, kt, lo:lo + width]
            assert lo >= 12 * P
            return xt3b[:, kt, lo - 12 * P:lo - 12 * P + width]

        # V projection with ones column: vc[st][:, h, 0:64]=V_h, [...,64]=1
        for st in range(NST):
            nc.vector.memset(vc[st][:, :, HD:HD + 1], 1.0)
        for stq in range(NST // 2):
            ps = psum.tile([P, 2 * DQ], F32, tag="sc", bufs=3)
            for half in range(2):
                st = 2 * stq + half
                sl = slice(half * DQ, (half + 1) * DQ)
                for kt in range(NKT):
                    nc.tensor.matmul(
                        ps[:, sl],
                        lhsT=xt_sl(kt, st * P, P),
                        rhs=wqkv_bf[kt][:, 2 * DQ:3 * DQ],
                        start=(kt == 0),
                        stop=(kt == NKT - 1),
                    )
            for half in range(2):
                st = 2 * stq + half
                src = ps[:, half * DQ:(half + 1) * DQ].rearrange(
                    "p (h e) -> p h e", h=NH
                )
                nc.vector.tensor_copy(vc[st][:, :, 0:HD], src)

        # Q^T / K^T projection bursts: one [128, 512] chunk = 8 matmuls
        # + 1 copy (~1.7us). Head-group 0's K (full S) + first q-chunk of Q
        # plus head-group 1's leading chunks are emitted in the head; the
        # rest drain inside the attention loop as 4-matmul half-bursts.
        def proj_burst(m, which, nq, width=512):
            dst = qt[m] if which == 0 else kt_sb[m]
            ps = psum.tile([P, width], F32, tag="sc", bufs=3, name="projps")
            for kt in range(NKT):
                nc.tensor.matmul(
                    ps[:],
                    lhsT=wqkv_bf[kt][:, which + m * P: which + (m + 1) * P],
                    rhs=xt_sl(kt, nq * width, width),
                    start=(kt == 0),
                    stop=(kt == NKT - 1),
                )
            nc.vector.tensor_copy(dst[:, nq * width:(nq + 1) * width], ps[:])

        for nq in range(4):
            proj_burst(0, DQ, nq)        # K^T head-group 0, full S
        proj_burst(0, 0, 0)              # Q^T head-group 0, q-chunk 0
        proj_burst(0, 0, 1)
        proj_burst(1, DQ, 0)             # head-group 1 leading chunks
        proj_burst(1, 0, 0)
        proj_burst(1, 0, 1)

    released = [False]

    def release_scope():
        if not released[0]:
            pscope.release()
            released[0] = True

    # In-loop injection is quarter-bursts: 2 accumulating matmuls (~0.4us
    # = one k-tile's PE slack vs the exp period) into one PSUM bank,
    # chained across quarters via an f32 SBUF stash on VectorE. Larger
    # units exceed the score-rotation's elasticity (ScalarE has at most
    # bufs-1 score tiles buffered) and stall the exp stream; 8-matmul
    # bursts demonstrably stall it by their full duration.
    stash = {}

    def proj_q(m, which, nq, kq):
        dst = qt[m] if which == 0 else kt_sb[m]
        key = (m, which, nq)
        ps = psum.tile([P, 512], F32, tag="sc", bufs=3, name="projps")
        for kt in range(2 * kq, 2 * kq + 2):
            nc.tensor.matmul(
                ps[:],
                lhsT=wqkv_bf[kt][:, which + m * P: which + (m + 1) * P],
                rhs=xt_sl(kt, nq * 512, 512),
                start=(kt == 2 * kq),
                stop=(kt == 2 * kq + 1),
            )
        if kq == 0:
            stg = pstage.tile([P, 512], F32, tag="pstg", bufs=3)
            nc.vector.tensor_copy(stg[:], ps[:])
            stash[key] = stg
        elif kq < 3:
            stg = stash[key]
            nc.vector.tensor_add(stg[:], ps[:], stg[:])
        else:
            nc.vector.tensor_add(
                dst[:, nq * 512:(nq + 1) * 512], ps[:], stash.pop(key)
            )

    # gated in-loop work queue: (gate_attend_idx, closure). Items pop in
    # order; a gated head blocks the slot (ordering preserves readiness).
    # Per head-group m: K n0 then Q qc0 halves then K n1-3 - the consuming
    # attend reads K column-blocks progressively (subtile deps) but needs
    # both Q halves of its q-chunk from k-tile 0.
    work_q = []

    def q_burst(gate, m, which, nq):
        for kq in range(4):
            work_q.append(
                (gate, lambda m=m, w=which, nq=nq, kq=kq: proj_q(m, w, nq, kq))
            )

    for nq in range(1, 4):
        q_burst(0, 1, DQ, nq)            # rest of K^T head-group 1
    for m in range(2, 4):
        q_burst(0, m, DQ, 0)
        q_burst(0, m, 0, 0)
        q_burst(0, m, 0, 1)
        for nq in range(1, 4):
            q_burst(0, m, DQ, nq)
    for m in range(4):
        q_burst(0, m, 0, 2)              # q-chunk-1 Q halves
        q_burst(0, m, 0, 3)
    work_q.append((0, lambda: release_scope()))

    # Out-projection quarters: head-group pair (c0,c1) is copied to a bf16
    # partial in SBUF, (c2,c3) is added on top (bf16 partials cost ~0.3%
    # of the output's own bf16 rounding, and the DMA is half the bytes).
    def outproj_q(st, half, cpair):
        sl = slice(half * 512, (half + 1) * 512)
        ps = psum.tile([P, 512], F32, tag="sc", bufs=3, name="ops")
        for c in (2 * cpair, 2 * cpair + 1):
            nc.tensor.matmul(
                ps[:],
                lhsT=ctxt[c][:, st * P:(st + 1) * P],
                rhs=wout_bf[c][:, sl],
                start=(c == 2 * cpair),
                stop=(c == 2 * cpair + 1),
            )
        if (half, cpair) == (0, 0):
            stash[("o", st)] = outsb_pool.tile(
                [P, D], BF16, tag="osb", bufs=10, name="osb"
            )
        osb = stash[("o", st)]
        if cpair == 0:
            nc.vector.tensor_copy(osb[:, sl], ps[:])
        else:
            nc.vector.tensor_add(osb[:, sl], ps[:], osb[:, sl])
        if (half, cpair) == (1, 1):
            stash.pop(("o", st))
            eng = (nc.sync, nc.scalar)[st % 2]
            eng.dma_start(out_ap[st * P:(st + 1) * P, :], osb[:])

    def outproj_tail(st):
        outproj_q(st, 0, 1)
        outproj_q(st, 1, 1)

    # out-projection first half: q-chunk-0 rows, ready once every head's
    # qc=0 normalization has drained (during attend 8's first k-tiles).
    # For the q-chunk-1 rows the (c0,c1) quarters only need heads 0-3,
    # normalized by attend 11 - they fill the otherwise-idle attends 12-15
    # and leave just the (c2,c3) quarters for the tail.
    for st in range(NST // 2):
        for half in range(2):
            for cpair in range(2):
                work_q.append(
                    (8, lambda st=st, h=half, c=cpair: outproj_q(st, h, c))
                )
    for st in range(NST // 2, NST):
        for half in range(2):
            work_q.append((12, lambda st=st, h=half: outproj_q(st, h, 0)))

    # ================= attention =================
    # deferred cross-attend work: closures drained 2-per-k-tile during the
    # first LAG k-tiles of the following attend (which has no ctx work)
    pending = []

    def drain(n):
        for _ in range(min(n, len(pending))):
            pending.pop(0)()

    def normalize(csb, m, r0, qc):
        """ctx^T[d,q] /= sum[q] (sums in row 64 of csb)."""
        q0 = qc * QC
        bc = psum.tile([HD, QC], F32, tag="sc", bufs=3)
        for half in range(2):
            sl = slice(half * 512, (half + 1) * 512)
            nc.tensor.matmul(
                bc[:, sl], lhsT=ones_rows[HD:HD + 1, :],
                rhs=csb[HD:HD + 1, sl],
                start=True, stop=True,
            )
        rec = small.tile([HD, QC], F32, tag="rec", bufs=2)
        nc.vector.reciprocal_approx_fast(rec[:], bc[:])
        nc.vector.tensor_mul(
            ctxt[m][r0:r0 + HD, q0:q0 + QC], csb[0:HD, :], rec[:]
        )

    def attend(aidx, h, qc):
        """One head x one 1024-q chunk; even head on partitions 0-63 of
        qt/kt_sb[m], odd head on 64-127."""
        q0 = qc * QC
        m, r0 = h // 2, HD * (h % 2)
        state = {}

        def emit_scores(kti):
            ps = psum.tile([P, QC], F32, tag="sc", bufs=3)
            for half in range(2):
                sl = slice(half * 512, (half + 1) * 512)
                qsl = slice(q0 + half * 512, q0 + (half + 1) * 512)
                nc.tensor.matmul(
                    ps[:, sl],
                    lhsT=kt_sb[m][r0:r0 + HD, kti * P:(kti + 1) * P],
                    rhs=qt[m][r0:r0 + HD, qsl],
                    start=True, stop=True,
                )
            return ps

        def emit_ctx(kti, pt):
            if kti == 0:
                state["ctx"] = psum.tile(
                    [HD + 1, QC], F32, tag="ctx", bufs=1, name="ctxp"
                )
            first = kti == 0
            last = kti == NST - 1
            for half in range(2):
                sl = slice(half * 512, (half + 1) * 512)
                nc.tensor.matmul(
                    state["ctx"][:, sl], lhsT=vc[kti][:, h, :],
                    rhs=pt[:, sl], start=first, stop=last,
                )

        pts = {}
        if aidx == 0:
            pop_kts = set(range(1, NST))
        elif aidx <= 5:
            pop_kts = {4, 6, 8, 9, 10, 11, 12, 14, 15}
        else:
            pop_kts = {4, 6, 8, 10, 12, 13, 14, 15}
        for kti in range(NST):
            ps = emit_scores(kti)
            if kti < LAG:
                drain(1)           # previous attend's tail ctx, 1 per k-tile
            else:
                emit_ctx(kti - LAG, pts.pop(kti - LAG))
                if kti < LAG + 2:
                    drain(1)       # evac (kt LAG) + normalize (kt LAG+1)
            if kti in pop_kts and work_q and work_q[0][0] <= aidx:
                work_q.pop(0)[1]()
            pt = pt_pool.tile([P, QC], BF16, tag="pt", bufs=12)
            nc.scalar.activation(pt[:], ps[:], Exp, scale=SCALE)
            pts[kti] = pt

        # tail: last LAG ctx tiles + PSUM evacuation + normalization are
        # deferred into the next attend's first k-tiles
        def tail_ctx(kti):
            def f():
                emit_ctx(kti, pts.pop(kti))
            return f

        for kti in range(NST - LAG, NST):
            pending.append(tail_ctx(kti))

        def evac():
            csb = small.tile([HD + 1, QC], BF16, tag="csb", bufs=4)
            nc.vector.tensor_copy(csb[:], state["ctx"][:])
            state["csb"] = csb

        pending.append(evac)
        pending.append(lambda: normalize(state["csb"], m, r0, qc))

    aidx = 0
    for qc in range(2):
        for h in range(NH):
            attend(aidx, h, qc)
            aidx += 1
    while work_q:
        work_q.pop(0)[1]()
    drain(len(pending))
    for st in range(NST // 2, NST):
        outproj_tail(st)


_CACHED = None


def _get_nc():
    global _CACHED
    if _CACHED is None:
        nc = bacc.Bacc(
            "TRN2", target_bir_lowering=False, debug=False, num_devices=8
        )
        hs = nc.dram_tensor("hs", [S, D], BF16, kind="ExternalInput").ap()
        wqkv = nc.dram_tensor("wqkv", [D, 3 * DQ], BF16, kind="ExternalInput").ap()
        wout = nc.dram_tensor("wout", [DQ, D], BF16, kind="ExternalInput").ap()
        out = nc.dram_tensor("out", [S, D], BF16, kind="ExternalOutput").ap()
        build_kernel(nc, out, hs, wqkv, wout)
        nc.compile()
        _CACHED = nc
    return _CACHED


def make_in_maps(hidden_states, w_qkv, w_out):
    in_maps = []
    for c in range(8):
        b, g = divmod(c, 2)
        cols = slice(g * DQ, (g + 1) * DQ)
        wq = w_qkv[:, 0 * D:1 * D][:, cols]
        wk = w_qkv[:, 1 * D:2 * D][:, cols]
        wv = w_qkv[:, 2 * D:3 * D][:, cols]
        bf = ml_dtypes.bfloat16
        in_maps.append({
            "hs": np.ascontiguousarray(hidden_states[b]).astype(bf),
            "wqkv": np.ascontiguousarray(
                np.concatenate([wq, wk, wv], axis=1)
            ).astype(bf),
            "wout": np.ascontiguousarray(
                w_out[g * DQ:(g + 1) * DQ, :]
            ).astype(bf),
        })
    return in_maps


def run(hidden_states, w_qkv, w_out, trace=False):
    nc = _get_nc()
    in_maps = make_in_maps(hidden_states, w_qkv, w_out)
    res = None
    last_err = None
    for _attempt in range(3):
        try:
            res = run_bass_kernel_spmd(
                nc, in_maps, core_ids=list(range(8)), trace=trace
            )
            break
        except Exception as e:  # transient NRT/device hiccups
            last_err = e
    if res is None:
        raise last_err
    out = np.empty((4, S, D), np.float32)
    for b in range(4):
        out[b] = (
            res.results[2 * b]["out"].astype(np.float32)
            + res.results[2 * b + 1]["out"].astype(np.float32)
        )
    return out, res


def kernel(hidden_states, w_qkv, w_out):
    out, _ = run(
        np.asarray(hidden_states), np.asarray(w_qkv), np.asarray(w_out)
    )
    return out


# revision 16
# speedup vs baseline: 1.0107x; 1.0077x over previous
"""AIMv2 attention (B=4, S=2048, D=1024, H=16, d=64) on 8 TRN2 NeuronCores.

Sharding: core c = (batch b = c//2, head-group g = c%2 of 8 heads).
Each core computes its batch's attention for its 8 heads plus the
out-projection partial sum over its heads' rows of w_out; the host adds
the two partials per batch (no on-device collectives needed).

Per-core kernel (all matmuls in bf16, fp32 accumulation; inputs are
pre-cast to bf16 on the host so no on-chip casts or fp32 staging):
  X^T via TensorE 128x128 transposes (keeps the PE HAM warm through the
  DMA phase); Q^T,K^T = Wq/k^T @ X^T so the score matmuls produce
  s_T[k, q] directly; softmax without max-subtraction (scores ~ N(0,1),
  exp never overflows fp32/bf16); V carries a ones column so
  ctx' = [V|1]^T @ P^T yields both ctx^T and the softmax denominators in
  one PSUM accumulation; normalization uses a K=1 broadcast matmul +
  reciprocal_approx_fast (exact reciprocal is 5x slower, and the approx
  op misbehaves at base_partition 64, hence the broadcast-first order).

  The attention loop processes ONE head x one 1024-q chunk per attend
  (16 attends of 16 k-tiles).  A single-head score tile is [128,1024]f32
  = 2 PSUM banks, so the score pipeline gets bufs=3 (6 banks) and the
  single ctx' accumulator [65,1024]f32 the remaining 2 banks.  The
  3-deep score rotation decouples the PE from ScalarE's exp stream (a
  2-deep rotation locksteps the two engines at the exp period and every
  injected matmul stretches the loop).  All remaining work - Q/K
  projection bursts for head-groups 1-3 and the first half of the
  out-projection - is a gated work queue drained one burst per few
  k-tiles inside the loop, sized to the PE's per-k-tile slack vs exp;
  cross-attend cleanup (last LAG ctx tiles, PSUM evacuation,
  normalization) drains during the first k-tiles of the next attend.
"""

import ml_dtypes
import numpy as np

import concourse.tile as tile
from concourse import bacc, mybir
from concourse.bass_utils import run_bass_kernel_spmd
from concourse.masks import make_identity

P = 128
S = 2048          # sequence length
D = 1024          # model dim
DQ = 512          # per-core qkv width (8 heads x 64)
HD = 64           # head dim
NH = 8            # heads per core
NKT = D // P      # 8 contraction tiles over D
NST = S // P      # 16 tiles over S
QC = 1024         # q chunk for attention inner loop
LAG = 6           # ctx matmul lag behind scores/exp in the pipeline
SCALE = 1.0 / 8.0  # 1/sqrt(64)

F32 = mybir.dt.float32
BF16 = mybir.dt.bfloat16


def build_kernel(nc, out_ap, hs_ap, wqkv_ap, wout_ap):
    import contextlib

    ctx = contextlib.ExitStack()
    with tile.TileContext(nc) as tc:
        with ctx:
            _body(ctx, tc, nc, out_ap, hs_ap, wqkv_ap, wout_ap)


def _body(ctx, tc, nc, out_ap, hs_ap, wqkv_ap, wout_ap):
    Exp = mybir.ActivationFunctionType.Exp

    persist = ctx.enter_context(tc.tile_pool(name="persist", bufs=1))
    psum = ctx.enter_context(tc.tile_pool(name="psum", bufs=1, space="PSUM"))

    # all-ones [128, 64] so a ones-row lhsT can be sliced at any base
    # partition (matmul requires lhsT/rhs base partitions to match)
    ones_rows = persist.tile([P, HD], BF16, name="ones_rows")
    nc.vector.memset(ones_rows[:], 1.0)

    wout_bf = []
    vc = [persist.tile([P, NH, HD + 1], BF16, name=f"vc{st}") for st in range(NST)]
    qt = [persist.tile([P, S], BF16, name=f"qt{m}") for m in range(4)]
    kt_sb = [persist.tile([P, S], BF16, name=f"kt{m}") for m in range(4)]
    ctxt = [persist.tile([P, S], BF16, name=f"ctxt{m}") for m in range(4)]

    pt_pool = ctx.enter_context(tc.tile_pool(name="pt", bufs=12))
    small = ctx.enter_context(tc.tile_pool(name="small", bufs=4))
    outsb_pool = ctx.enter_context(tc.tile_pool(name="outsb", bufs=3))
    pstage = ctx.enter_context(tc.tile_pool(name="pstage", bufs=3))

    # ================= head: loads + all projections =================
    # Inputs arrive pre-cast to bf16 (host-side), so weights DMA straight
    # into their bf16 tiles; X^T is built by TensorE 128x128 transposes.
    # proj_scope (X^T + qkv weights) is released once the last queued
    # projection burst has been emitted, mid-attention.
    pscope = tc.alloc_tile_pool(name="proj_scope", bufs=1)
    if True:
        # sts 0-11 transpose on TensorE into xt3a; sts 12-15 go through the
        # XBAR DMA path (single engine - two-engine xbar use corrupts) into a
        # SEPARATE tensor so the two paths share no WAW/WAR dependences.
        xt3a = pscope.tile([P, NKT, 12 * P], BF16, name="xt3a")
        xt3b = pscope.tile([P, NKT, 4 * P], BF16, name="xt3b")
        wqkv_bf = []

        identity = pscope.tile([P, P], BF16, name="identity")
        make_identity(nc, identity[:])

        # free-running warm-up burst: ~4us of tiny matmuls flips the PE HAM
        # to K=8/8 before the transpose/projection phase so the (PE-bound)
        # head doesn't run at 1.2 GHz when the kernel lands on a cold HAM
        # window; kept alive through DCE via the 0-scaled add below
        warm_ps = psum.tile([HD, HD], F32, tag="ctx", bufs=1, name="warm_ps")
        N_WARM = 48
        for wi in range(N_WARM):
            nc.tensor.matmul(
                warm_ps[:], lhsT=ones_rows[0:HD, :], rhs=ones_rows[0:HD, :],
                start=(wi == 0), stop=(wi == N_WARM - 1),
            )
        warmsb = pscope.tile([HD, HD], F32, name="warmsb")
        nc.vector.tensor_scalar_mul(warmsb[:], warm_ps[:], 0.0)
        nc.vector.tensor_add(ones_rows[0:HD, :], ones_rows[0:HD, :], warmsb[:])

        # weight DMAs ride the sync queue; the X row-tiles (which gate the
        # PE's transpose work) go first on the scalar queue, wout last
        for kt in range(NKT):
            wb = pscope.tile([P, 3 * DQ], BF16, name=f"wqkv_bf{kt}")
            nc.sync.dma_start(wb[:], wqkv_ap[kt * P:(kt + 1) * P, :])
            wqkv_bf.append(wb)

        # X: load bf16 row-tiles, transpose 128x128 blocks on TensorE (PE is
        # otherwise idle here and this keeps HAM warm), evacuate per-row-tile
        with tc.tile_pool(name="stage", bufs=4) as stage:
            for st in range(NST):
                xb = stage.tile([P, D], BF16, tag="xbf", bufs=4)
                nc.scalar.dma_start(xb[:], hs_ap[st * P:(st + 1) * P, :])
                if st < 12:
                    ps_t = psum.tile([P, D], BF16, tag="sc", bufs=2, name="ps_t")
                    for dt in range(NKT):
                        nc.tensor.transpose(
                            ps_t[:, dt * P:(dt + 1) * P],
                            xb[:, dt * P:(dt + 1) * P],
                            identity[:],
                        )
                    nc.vector.tensor_copy(
                        xt3a[:, :, st * P:(st + 1) * P],
                        ps_t[:].rearrange("p (h e) -> p h e", h=NKT),
                    )
                else:
                    nc.sync.dma_start_transpose(
                        xt3b[:, :, (st - 12) * P:(st - 11) * P], xb[:]
                    )

        for i in range(DQ // P):
            wb = persist.tile([P, D], BF16, name=f"wout_bf{i}")
            nc.scalar.dma_start(wb[:], wout_ap[i * P:(i + 1) * P, :])
            wout_bf.append(wb)

        def xt_sl(kt, lo, width):
            # column slice [lo, lo+width) of X^T row-block kt
            if lo + width <= 12 * P:
                return xt3a[:, kt, lo:lo + width]
            assert lo >= 12 * P
            return xt3b[:, kt, lo - 12 * P:lo - 12 * P + width]

        # V projection with ones column: vc[st][:, h, 0:64]=V_h, [...,64]=1
        for st in range(NST):
            nc.vector.memset(vc[st][:, :, HD:HD + 1], 1.0)
        for stq in range(NST // 2):
            ps = psum.tile([P, 2 * DQ], F32, tag="sc", bufs=2)
            for half in range(2):
                st = 2 * stq + half
                sl = slice(half * DQ, (half + 1) * DQ)
                for kt in range(NKT):
                    nc.tensor.matmul(
                        ps[:, sl],
                        lhsT=xt_sl(kt, st * P, P),
                        rhs=wqkv_bf[kt][:, 2 * DQ:3 * DQ],
                        start=(kt == 0),
                        stop=(kt == NKT - 1),
                    )
            for half in range(2):
                st = 2 * stq + half
                src = ps[:, half * DQ:(half + 1) * DQ].rearrange(
                    "p (h e) -> p h e", h=NH
                )
                nc.vector.tensor_copy(vc[st][:, :, 0:HD], src)

        # Q^T / K^T projection bursts: one [128, 512] chunk = 8 matmuls
        # + 1 copy (~1.7us). Head-group 0's K (full S) + first q-chunk of Q
        # plus head-group 1's leading chunks are emitted in the head; the
        # rest drain inside the attention loop as 4-matmul half-bursts.
        def proj_burst(m, which, nq, width=512):
            dst = qt[m] if which == 0 else kt_sb[m]
            ps = psum.tile([P, width], F32, tag="u", bufs=2, name="projps")
            for kt in range(NKT):
                nc.tensor.matmul(
                    ps[:],
                    lhsT=wqkv_bf[kt][:, which + m * P: which + (m + 1) * P],
                    rhs=xt_sl(kt, nq * width, width),
                    start=(kt == 0),
                    stop=(kt == NKT - 1),
                )
            nc.vector.tensor_copy(dst[:, nq * width:(nq + 1) * width], ps[:])

        for nq in range(4):
            proj_burst(0, DQ, nq)        # K^T head-group 0, full S
        proj_burst(0, 0, 0)              # Q^T head-group 0, q-chunk 0
        proj_burst(0, 0, 1)
        proj_burst(1, DQ, 0)             # head-group 1 leading chunks
        proj_burst(1, 0, 0)
        proj_burst(1, 0, 1)

    released = [False]

    def release_scope():
        if not released[0]:
            pscope.release()
            released[0] = True

    # In-loop injection is quarter-bursts: 2 accumulating matmuls (~0.4us
    # = one k-tile's PE slack vs the exp period) into one PSUM bank,
    # chained across quarters via an f32 SBUF stash on VectorE. Larger
    # units exceed the score-rotation's elasticity (ScalarE has at most
    # bufs-1 score tiles buffered) and stall the exp stream; 8-matmul
    # bursts demonstrably stall it by their full duration.
    stash = {}

    def proj_q(m, which, nq, kq):
        dst = qt[m] if which == 0 else kt_sb[m]
        key = (m, which, nq)
        ps = psum.tile([P, 512], F32, tag="u", bufs=2, name="projps")
        for kt in range(2 * kq, 2 * kq + 2):
            nc.tensor.matmul(
                ps[:],
                lhsT=wqkv_bf[kt][:, which + m * P: which + (m + 1) * P],
                rhs=xt_sl(kt, nq * 512, 512),
                start=(kt == 2 * kq),
                stop=(kt == 2 * kq + 1),
            )
        if kq == 0:
            stg = pstage.tile([P, 512], F32, tag="pstg", bufs=3)
            nc.vector.tensor_copy(stg[:], ps[:])
            stash[key] = stg
        elif kq < 3:
            stg = stash[key]
            nc.vector.tensor_add(stg[:], ps[:], stg[:])
        else:
            nc.vector.tensor_add(
                dst[:, nq * 512:(nq + 1) * 512], ps[:], stash.pop(key)
            )

    # gated in-loop work queue: (gate_attend_idx, closure). Items pop in
    # order; a gated head blocks the slot (ordering preserves readiness).
    # Per head-group m: K n0 then Q qc0 halves then K n1-3 - the consuming
    # attend reads K column-blocks progressively (subtile deps) but needs
    # both Q halves of its q-chunk from k-tile 0.
    work_q = []

    def q_burst(gate, m, which, nq):
        for kq in range(4):
            work_q.append(
                (gate, lambda m=m, w=which, nq=nq, kq=kq: proj_q(m, w, nq, kq))
            )

    for nq in range(1, 4):
        q_burst(0, 1, DQ, nq)            # rest of K^T head-group 1
    for m in range(2, 4):
        q_burst(0, m, DQ, 0)
        q_burst(0, m, 0, 0)
        q_burst(0, m, 0, 1)
        for nq in range(1, 4):
            q_burst(0, m, DQ, nq)
    for m in range(4):
        q_burst(0, m, 0, 2)              # q-chunk-1 Q halves
        q_burst(0, m, 0, 3)
    work_q.append((0, lambda: release_scope()))

    # Out-projection quarters: head-group pair (c0,c1) is copied to a bf16
    # partial in SBUF, (c2,c3) is added on top (bf16 partials cost ~0.3%
    # of the output's own bf16 rounding, and the DMA is half the bytes).
    def outproj_q(st, half, cpair):
        sl = slice(half * 512, (half + 1) * 512)
        ps = psum.tile([P, 512], F32, tag="u", bufs=2, name="ops")
        for c in (2 * cpair, 2 * cpair + 1):
            nc.tensor.matmul(
                ps[:],
                lhsT=ctxt[c][:, st * P:(st + 1) * P],
                rhs=wout_bf[c][:, sl],
                start=(c == 2 * cpair),
                stop=(c == 2 * cpair + 1),
            )
        if (half, cpair) == (0, 0):
            stash[("o", st)] = outsb_pool.tile(
                [P, D], BF16, tag="osb", bufs=10, name="osb"
            )
        osb = stash[("o", st)]
        if cpair == 0:
            nc.vector.tensor_copy(osb[:, sl], ps[:])
        else:
            nc.vector.tensor_add(osb[:, sl], ps[:], osb[:, sl])
        if (half, cpair) == (1, 1):
            stash.pop(("o", st))
            eng = (nc.sync, nc.scalar)[st % 2]
            eng.dma_start(out_ap[st * P:(st + 1) * P, :], osb[:])

    def outproj_tail(st):
        outproj_q(st, 0, 1)
        outproj_q(st, 1, 1)

    # out-projection first half: q-chunk-0 rows, ready once every head's
    # qc=0 normalization has drained (during attend 8's first k-tiles).
    # For the q-chunk-1 rows the (c0,c1) quarters only need heads 0-3,
    # normalized by attend 11 - they fill the otherwise-idle attends 12-15
    # and leave just the (c2,c3) quarters for the tail.
    for st in range(NST // 2):
        for half in range(2):
            for cpair in range(2):
                work_q.append(
                    (8, lambda st=st, h=half, c=cpair: outproj_q(st, h, c))
                )
    for st in range(NST // 2, NST):
        for half in range(2):
            work_q.append((12, lambda st=st, h=half: outproj_q(st, h, 0)))

    # ================= attention =================
    # deferred cross-attend work: closures drained 2-per-k-tile during the
    # first LAG k-tiles of the following attend (which has no ctx work)
    pending = []

    def drain(n):
        for _ in range(min(n, len(pending))):
            pending.pop(0)()

    def normalize(csb, m, r0, qc):
        """ctx^T[d,q] /= sum[q] (sums in row 64 of csb)."""
        q0 = qc * QC
        bc = psum.tile([HD, QC], F32, tag="sc", bufs=2)
        for half in range(2):
            sl = slice(half * 512, (half + 1) * 512)
            nc.tensor.matmul(
                bc[:, sl], lhsT=ones_rows[HD:HD + 1, :],
                rhs=csb[HD:HD + 1, sl],
                start=True, stop=True,
            )
        rec = small.tile([HD, QC], F32, tag="rec", bufs=2)
        nc.vector.reciprocal_approx_fast(rec[:], bc[:])
        nc.vector.tensor_mul(
            ctxt[m][r0:r0 + HD, q0:q0 + QC], csb[0:HD, :], rec[:]
        )

    def attend(aidx, h, qc):
        """One head x one 1024-q chunk; even head on partitions 0-63 of
        qt/kt_sb[m], odd head on 64-127."""
        q0 = qc * QC
        m, r0 = h // 2, HD * (h % 2)
        state = {}

        def emit_scores(kti):
            ps = psum.tile([P, QC], F32, tag="sc", bufs=2)
            for half in range(2):
                sl = slice(half * 512, (half + 1) * 512)
                qsl = slice(q0 + half * 512, q0 + (half + 1) * 512)
                nc.tensor.matmul(
                    ps[:, sl],
                    lhsT=kt_sb[m][r0:r0 + HD, kti * P:(kti + 1) * P],
                    rhs=qt[m][r0:r0 + HD, qsl],
                    start=True, stop=True,
                )
            return ps

        def emit_ctx(kti, pt):
            if kti == 0:
                state["ctx"] = psum.tile(
                    [HD + 1, QC], F32, tag="ctx", bufs=1, name="ctxp"
                )
            first = kti == 0
            last = kti == NST - 1
            for half in range(2):
                sl = slice(half * 512, (half + 1) * 512)
                nc.tensor.matmul(
                    state["ctx"][:, sl], lhsT=vc[kti][:, h, :],
                    rhs=pt[:, sl], start=first, stop=last,
                )

        pts = {}
        if aidx == 0:
            pop_kts = set(range(1, NST))
        elif aidx <= 5:
            pop_kts = {4, 6, 8, 9, 10, 11, 12, 14, 15}
        else:
            pop_kts = {4, 6, 8, 10, 12, 13, 14, 15}
        for kti in range(NST):
            ps = emit_scores(kti)
            if kti < LAG:
                drain(1)           # previous attend's tail ctx, 1 per k-tile
            else:
                emit_ctx(kti - LAG, pts.pop(kti - LAG))
                if kti < LAG + 2:
                    drain(1)       # evac (kt LAG) + normalize (kt LAG+1)
            if kti in pop_kts and work_q and work_q[0][0] <= aidx:
                work_q.pop(0)[1]()
            pt = pt_pool.tile([P, QC], BF16, tag="pt", bufs=12)
            nc.scalar.activation(pt[:], ps[:], Exp, scale=SCALE)
            pts[kti] = pt

        # tail: last LAG ctx tiles + PSUM evacuation + normalization are
        # deferred into the next attend's first k-tiles
        def tail_ctx(kti):
            def f():
                emit_ctx(kti, pts.pop(kti))
            return f

        for kti in range(NST - LAG, NST):
            pending.append(tail_ctx(kti))

        def evac():
            csb = small.tile([HD + 1, QC], BF16, tag="csb", bufs=4)
            nc.vector.tensor_copy(csb[:], state["ctx"][:])
            state["csb"] = csb

        pending.append(evac)
        pending.append(lambda: normalize(state["csb"], m, r0, qc))

    aidx = 0
    for qc in range(2):
        for h in range(NH):
            attend(aidx, h, qc)
            aidx += 1
    while work_q:
        work_q.pop(0)[1]()
    drain(len(pending))
    for st in range(NST // 2, NST):
        outproj_tail(st)


_CACHED = None


def _get_nc():
    global _CACHED
    if _CACHED is None:
        nc = bacc.Bacc(
            "TRN2", target_bir_lowering=False, debug=False, num_devices=8
        )
        hs = nc.dram_tensor("hs", [S, D], BF16, kind="ExternalInput").ap()
        wqkv = nc.dram_tensor("wqkv", [D, 3 * DQ], BF16, kind="ExternalInput").ap()
        wout = nc.dram_tensor("wout", [DQ, D], BF16, kind="ExternalInput").ap()
        out = nc.dram_tensor("out", [S, D], BF16, kind="ExternalOutput").ap()
        build_kernel(nc, out, hs, wqkv, wout)
        nc.compile()
        _CACHED = nc
    return _CACHED


def make_in_maps(hidden_states, w_qkv, w_out):
    in_maps = []
    for c in range(8):
        b, g = divmod(c, 2)
        cols = slice(g * DQ, (g + 1) * DQ)
        wq = w_qkv[:, 0 * D:1 * D][:, cols]
        wk = w_qkv[:, 1 * D:2 * D][:, cols]
        wv = w_qkv[:, 2 * D:3 * D][:, cols]
        bf = ml_dtypes.bfloat16
        in_maps.append({
            "hs": np.ascontiguousarray(hidden_states[b]).astype(bf),
            "wqkv": np.ascontiguousarray(
                np.concatenate([wq, wk, wv], axis=1)
            ).astype(bf),
            "wout": np.ascontiguousarray(
                w_out[g * DQ:(g + 1) * DQ, :]
            ).astype(bf),
        })
    return in_maps


def run(hidden_states, w_qkv, w_out, trace=False):
    nc = _get_nc()
    in_maps = make_in_maps(hidden_states, w_qkv, w_out)
    res = None
    last_err = None
    for _attempt in range(3):
        try:
            res = run_bass_kernel_spmd(
                nc, in_maps, core_ids=list(range(8)), trace=trace
            )
            break
        except Exception as e:  # transient NRT/device hiccups
            last_err = e
    if res is None:
        raise last_err
    out = np.empty((4, S, D), np.float32)
    for b in range(4):
        out[b] = (
            res.results[2 * b]["out"].astype(np.float32)
            + res.results[2 * b + 1]["out"].astype(np.float32)
        )
    return out, res


def kernel(hidden_states, w_qkv, w_out):
    out, _ = run(
        np.asarray(hidden_states), np.asarray(w_qkv), np.asarray(w_out)
    )
    return out


# revision 18
# speedup vs baseline: 1.0112x; 1.0006x over previous
"""AIMv2 attention (B=4, S=2048, D=1024, H=16, d=64) on 8 TRN2 NeuronCores.

Sharding: core c = (batch b = c//2, head-group g = c%2 of 8 heads).
Each core computes its batch's attention for its 8 heads plus the
out-projection partial sum over its heads' rows of w_out; the host adds
the two partials per batch (no on-device collectives needed).

Per-core kernel (all matmuls in bf16, fp32 accumulation; inputs are
pre-cast to bf16 on the host so no on-chip casts or fp32 staging):
  X^T via TensorE 128x128 transposes (keeps the PE HAM warm through the
  DMA phase); Q^T,K^T = Wq/k^T @ X^T so the score matmuls produce
  s_T[k, q] directly; softmax without max-subtraction (scores ~ N(0,1),
  exp never overflows fp32/bf16); V carries a ones column so
  ctx' = [V|1]^T @ P^T yields both ctx^T and the softmax denominators in
  one PSUM accumulation; normalization uses a K=1 broadcast matmul +
  reciprocal_approx_fast (exact reciprocal is 5x slower, and the approx
  op misbehaves at base_partition 64, hence the broadcast-first order).

  The attention loop processes ONE head x one 1024-q chunk per attend
  (16 attends of 16 k-tiles).  A single-head score tile is [128,1024]f32
  = 2 PSUM banks, so the score pipeline gets bufs=3 (6 banks) and the
  single ctx' accumulator [65,1024]f32 the remaining 2 banks.  The
  3-deep score rotation decouples the PE from ScalarE's exp stream (a
  2-deep rotation locksteps the two engines at the exp period and every
  injected matmul stretches the loop).  All remaining work - Q/K
  projection bursts for head-groups 1-3 and the first half of the
  out-projection - is a gated work queue drained one burst per few
  k-tiles inside the loop, sized to the PE's per-k-tile slack vs exp;
  cross-attend cleanup (last LAG ctx tiles, PSUM evacuation,
  normalization) drains during the first k-tiles of the next attend.
"""

import ml_dtypes
import numpy as np

import concourse.tile as tile
from concourse import bacc, mybir
from concourse.bass_utils import run_bass_kernel_spmd
from concourse.masks import make_identity

P = 128
S = 2048          # sequence length
D = 1024          # model dim
DQ = 512          # per-core qkv width (8 heads x 64)
HD = 64           # head dim
NH = 8            # heads per core
NKT = D // P      # 8 contraction tiles over D
NST = S // P      # 16 tiles over S
QC = 1024         # q chunk for attention inner loop
LAG = 6           # ctx matmul lag behind scores/exp in the pipeline
SCALE = 1.0 / 8.0  # 1/sqrt(64)

F32 = mybir.dt.float32
BF16 = mybir.dt.bfloat16


def build_kernel(nc, out_ap, hs_ap, wqkv_ap, wout_ap):
    import contextlib

    ctx = contextlib.ExitStack()
    with tile.TileContext(nc) as tc:
        with ctx:
            _body(ctx, tc, nc, out_ap, hs_ap, wqkv_ap, wout_ap)


def _body(ctx, tc, nc, out_ap, hs_ap, wqkv_ap, wout_ap):
    Exp = mybir.ActivationFunctionType.Exp

    persist = ctx.enter_context(tc.tile_pool(name="persist", bufs=1))
    psum = ctx.enter_context(tc.tile_pool(name="psum", bufs=1, space="PSUM"))

    # all-ones [128, 64] so a ones-row lhsT can be sliced at any base
    # partition (matmul requires lhsT/rhs base partitions to match)
    ones_rows = persist.tile([P, HD], BF16, name="ones_rows")
    nc.vector.memset(ones_rows[:], 1.0)

    wout_bf = []
    vc = [persist.tile([P, NH, HD + 1], BF16, name=f"vc{st}") for st in range(NST)]
    qt = [persist.tile([P, S], BF16, name=f"qt{m}") for m in range(4)]
    kt_sb = [persist.tile([P, S], BF16, name=f"kt{m}") for m in range(4)]
    ctxt = [persist.tile([P, S], BF16, name=f"ctxt{m}") for m in range(4)]

    pt_pool = ctx.enter_context(tc.tile_pool(name="pt", bufs=12))
    small = ctx.enter_context(tc.tile_pool(name="small", bufs=4))
    outsb_pool = ctx.enter_context(tc.tile_pool(name="outsb", bufs=3))
    pstage = ctx.enter_context(tc.tile_pool(name="pstage", bufs=3))

    # ================= head: loads + all projections =================
    # Inputs arrive pre-cast to bf16 (host-side), so weights DMA straight
    # into their bf16 tiles; X^T is built by TensorE 128x128 transposes.
    # proj_scope (X^T + qkv weights) is released once the last queued
    # projection burst has been emitted, mid-attention.
    pscope = tc.alloc_tile_pool(name="proj_scope", bufs=1)
    if True:
        # sts 0-11 transpose on TensorE into xt3a; sts 12-15 go through the
        # XBAR DMA path (single engine - two-engine xbar use corrupts) into a
        # SEPARATE tensor so the two paths share no WAW/WAR dependences.
        xt3a = pscope.tile([P, NKT, 12 * P], BF16, name="xt3a")
        xt3b = pscope.tile([P, NKT, 4 * P], BF16, name="xt3b")
        wqkv_bf = []

        identity = pscope.tile([P, P], BF16, name="identity")
        make_identity(nc, identity[:])

        # free-running warm-up burst: ~4us of tiny matmuls flips the PE HAM
        # to K=8/8 before the transpose/projection phase so the (PE-bound)
        # head doesn't run at 1.2 GHz when the kernel lands on a cold HAM
        # window; kept alive through DCE via the 0-scaled add below
        warm_ps = psum.tile([HD, HD], F32, tag="ctx", bufs=1, name="warm_ps")
        N_WARM = 48
        for wi in range(N_WARM):
            nc.tensor.matmul(
                warm_ps[:], lhsT=ones_rows[0:HD, :], rhs=ones_rows[0:HD, :],
                start=(wi == 0), stop=(wi == N_WARM - 1),
            )
        warmsb = pscope.tile([HD, HD], F32, name="warmsb")
        nc.vector.tensor_scalar_mul(warmsb[:], warm_ps[:], 0.0)
        nc.vector.tensor_add(ones_rows[0:HD, :], ones_rows[0:HD, :], warmsb[:])

        # weight DMAs ride the sync queue; the X row-tiles (which gate the
        # PE's transpose work) go first on the scalar queue, wout last
        for kt in range(NKT):
            wb = pscope.tile([P, 3 * DQ], BF16, name=f"wqkv_bf{kt}")
            nc.sync.dma_start(wb[:], wqkv_ap[kt * P:(kt + 1) * P, :])
            wqkv_bf.append(wb)

        # X: load bf16 row-tiles, transpose 128x128 blocks on TensorE (PE is
        # otherwise idle here and this keeps HAM warm), evacuate per-row-tile
        with tc.tile_pool(name="stage", bufs=4) as stage:
            for st in range(NST):
                xb = stage.tile([P, D], BF16, tag="xbf", bufs=4)
                nc.scalar.dma_start(xb[:], hs_ap[st * P:(st + 1) * P, :])
                if st < 12:
                    ps_t = psum.tile([P, D], BF16, tag="sc", bufs=2, name="ps_t")
                    for dt in range(NKT):
                        nc.tensor.transpose(
                            ps_t[:, dt * P:(dt + 1) * P],
                            xb[:, dt * P:(dt + 1) * P],
                            identity[:],
                        )
                    nc.vector.tensor_copy(
                        xt3a[:, :, st * P:(st + 1) * P],
                        ps_t[:].rearrange("p (h e) -> p h e", h=NKT),
                    )
                else:
                    nc.sync.dma_start_transpose(
                        xt3b[:, :, (st - 12) * P:(st - 11) * P], xb[:]
                    )

        for i in range(DQ // P):
            wb = persist.tile([P, D], BF16, name=f"wout_bf{i}")
            nc.scalar.dma_start(wb[:], wout_ap[i * P:(i + 1) * P, :])
            wout_bf.append(wb)

        def xt_sl(kt, lo, width):
            # column slice [lo, lo+width) of X^T row-block kt
            if lo + width <= 12 * P:
                return xt3a[:, kt, lo:lo + width]
            assert lo >= 12 * P
            return xt3b[:, kt, lo - 12 * P:lo - 12 * P + width]

        # V projection with ones column: vc[st][:, h, 0:64]=V_h, [...,64]=1
        for st in range(NST):
            nc.vector.memset(vc[st][:, :, HD:HD + 1], 1.0)
        for stq in range(NST // 2):
            ps = psum.tile([P, 2 * DQ], F32, tag="sc", bufs=2)
            for half in range(2):
                st = 2 * stq + half
                sl = slice(half * DQ, (half + 1) * DQ)
                for kt in range(NKT):
                    nc.tensor.matmul(
                        ps[:, sl],
                        lhsT=xt_sl(kt, st * P, P),
                        rhs=wqkv_bf[kt][:, 2 * DQ:3 * DQ],
                        start=(kt == 0),
                        stop=(kt == NKT - 1),
                    )
            for half in range(2):
                st = 2 * stq + half
                src = ps[:, half * DQ:(half + 1) * DQ].rearrange(
                    "p (h e) -> p h e", h=NH
                )
                nc.vector.tensor_copy(vc[st][:, :, 0:HD], src)

        # Q^T / K^T projection bursts: one [128, 512] chunk = 8 matmuls
        # + 1 copy (~1.7us). Head-group 0's K (full S) + first q-chunk of Q
        # plus head-group 1's leading chunks are emitted in the head; the
        # rest drain inside the attention loop as 4-matmul half-bursts.
        def proj_burst(m, which, nq, width=512):
            dst = qt[m] if which == 0 else kt_sb[m]
            ps = psum.tile([P, width], F32, tag="u", bufs=2, name="projps")
            for kt in range(NKT):
                nc.tensor.matmul(
                    ps[:],
                    lhsT=wqkv_bf[kt][:, which + m * P: which + (m + 1) * P],
                    rhs=xt_sl(kt, nq * width, width),
                    start=(kt == 0),
                    stop=(kt == NKT - 1),
                )
            nc.vector.tensor_copy(dst[:, nq * width:(nq + 1) * width], ps[:])

        for nq in range(4):
            proj_burst(0, DQ, nq)        # K^T head-group 0, full S
        proj_burst(0, 0, 0)              # Q^T head-group 0, q-chunk 0
        proj_burst(0, 0, 1)
        proj_burst(1, DQ, 0)             # head-group 1 leading chunks
        proj_burst(1, 0, 0)
        proj_burst(1, 0, 1)

    released = [False]

    def release_scope():
        if not released[0]:
            pscope.release()
            released[0] = True

    # In-loop injection is quarter-bursts: 2 accumulating matmuls (~0.4us
    # = one k-tile's PE slack vs the exp period) into one PSUM bank,
    # chained across quarters via an f32 SBUF stash on VectorE. Larger
    # units exceed the score-rotation's elasticity (ScalarE has at most
    # bufs-1 score tiles buffered) and stall the exp stream; 8-matmul
    # bursts demonstrably stall it by their full duration.
    stash = {}

    def proj_q(m, which, nq, kq):
        dst = qt[m] if which == 0 else kt_sb[m]
        key = (m, which, nq)
        ps = psum.tile([P, 512], F32, tag="u", bufs=2, name="projps")
        for kt in range(2 * kq, 2 * kq + 2):
            nc.tensor.matmul(
                ps[:],
                lhsT=wqkv_bf[kt][:, which + m * P: which + (m + 1) * P],
                rhs=xt_sl(kt, nq * 512, 512),
                start=(kt == 2 * kq),
                stop=(kt == 2 * kq + 1),
            )
        if kq == 0:
            stg = pstage.tile([P, 512], F32, tag="pstg", bufs=3)
            nc.vector.tensor_copy(stg[:], ps[:])
            stash[key] = stg
        elif kq < 3:
            stg = stash[key]
            nc.vector.tensor_add(stg[:], ps[:], stg[:])
        else:
            nc.vector.tensor_add(
                dst[:, nq * 512:(nq + 1) * 512], ps[:], stash.pop(key)
            )

    # gated in-loop work queue: (gate_attend_idx, closure). Items pop in
    # order; a gated head blocks the slot (ordering preserves readiness).
    # Per head-group m: K n0 then Q qc0 halves then K n1-3 - the consuming
    # attend reads K column-blocks progressively (subtile deps) but needs
    # both Q halves of its q-chunk from k-tile 0.
    work_q = []

    def q_burst(gate, m, which, nq):
        for kq in range(4):
            work_q.append(
                (gate, lambda m=m, w=which, nq=nq, kq=kq: proj_q(m, w, nq, kq))
            )

    for nq in range(1, 4):
        q_burst(0, 1, DQ, nq)            # rest of K^T head-group 1
    for m in range(2, 4):
        q_burst(0, m, DQ, 0)
        q_burst(0, m, 0, 0)
        q_burst(0, m, 0, 1)
        for nq in range(1, 4):
            q_burst(0, m, DQ, nq)
    for m in range(4):
        q_burst(0, m, 0, 2)              # q-chunk-1 Q halves
        q_burst(0, m, 0, 3)
    work_q.append((0, lambda: release_scope()))

    # Out-projection quarters: head-group pair (c0,c1) is copied to a bf16
    # partial in SBUF, (c2,c3) is added on top (bf16 partials cost ~0.3%
    # of the output's own bf16 rounding, and the DMA is half the bytes).
    def outproj_q(st, half, cpair):
        sl = slice(half * 512, (half + 1) * 512)
        ps = psum.tile([P, 512], F32, tag="u", bufs=2, name="ops")
        for c in (2 * cpair, 2 * cpair + 1):
            nc.tensor.matmul(
                ps[:],
                lhsT=ctxt[c][:, st * P:(st + 1) * P],
                rhs=wout_bf[c][:, sl],
                start=(c == 2 * cpair),
                stop=(c == 2 * cpair + 1),
            )
        if (half, cpair) == (0, 0):
            stash[("o", st)] = outsb_pool.tile(
                [P, D], BF16, tag="osb", bufs=10, name="osb"
            )
        osb = stash[("o", st)]
        if cpair == 0:
            nc.vector.tensor_copy(osb[:, sl], ps[:])
        else:
            nc.vector.tensor_add(osb[:, sl], ps[:], osb[:, sl])
        if (half, cpair) == (1, 1):
            stash.pop(("o", st))
            eng = (nc.sync, nc.scalar)[st % 2]
            eng.dma_start(out_ap[st * P:(st + 1) * P, :], osb[:])

    def outproj_tail(st):
        outproj_q(st, 0, 1)
        outproj_q(st, 1, 1)

    # out-projection first half: q-chunk-0 rows, ready once every head's
    # qc=0 normalization has drained (during attend 8's first k-tiles).
    # For the q-chunk-1 rows the (c0,c1) quarters only need heads 0-3,
    # normalized by attend 11 - they fill the otherwise-idle attends 12-15
    # and leave just the (c2,c3) quarters for the tail.
    for st in range(NST // 2):
        for half in range(2):
            for cpair in range(2):
                work_q.append(
                    (8, lambda st=st, h=half, c=cpair: outproj_q(st, h, c))
                )
    for st in range(NST // 2, NST):
        for half in range(2):
            work_q.append((12, lambda st=st, h=half: outproj_q(st, h, 0)))

    # ================= attention =================
    # One global FIFO work-stream, popped EXACTLY one ~0.4us item per
    # k-tile at every k-tile: previous-attend tail ctx / evacuation /
    # normalization, then injected quarter-bursts. Even pacing holds the
    # PE at ~1.13us/kt, just above the 1.085us exp period, so neither
    # engine accumulates a deficit the shallow PSUM rotation can't buffer
    # (clustered draining/injection demonstrably stalls one side or the
    # other). Items carry an attend-index gate; a gated head blocks the
    # stream to preserve ordering.
    def drain(n):
        for _ in range(min(n, len(work_q))):
            work_q.pop(0)[1]()

    def normalize(csb, m, r0, qc):
        """ctx^T[d,q] /= sum[q] (sums in row 64 of csb)."""
        q0 = qc * QC
        bc = psum.tile([HD, QC], F32, tag="sc", bufs=2)
        for half in range(2):
            sl = slice(half * 512, (half + 1) * 512)
            nc.tensor.matmul(
                bc[:, sl], lhsT=ones_rows[HD:HD + 1, :],
                rhs=csb[HD:HD + 1, sl],
                start=True, stop=True,
            )
        rec = small.tile([HD, QC], F32, tag="rec", bufs=2)
        nc.vector.reciprocal_approx_fast(rec[:], bc[:])
        nc.vector.tensor_mul(
            ctxt[m][r0:r0 + HD, q0:q0 + QC], csb[0:HD, :], rec[:]
        )

    def attend(aidx, h, qc):
        """One head x one 1024-q chunk; even head on partitions 0-63 of
        qt/kt_sb[m], odd head on 64-127."""
        q0 = qc * QC
        m, r0 = h // 2, HD * (h % 2)
        state = {}

        def emit_scores(kti):
            ps = psum.tile([P, QC], F32, tag="sc", bufs=2)
            for half in range(2):
                sl = slice(half * 512, (half + 1) * 512)
                qsl = slice(q0 + half * 512, q0 + (half + 1) * 512)
                nc.tensor.matmul(
                    ps[:, sl],
                    lhsT=kt_sb[m][r0:r0 + HD, kti * P:(kti + 1) * P],
                    rhs=qt[m][r0:r0 + HD, qsl],
                    start=True, stop=True,
                )
            return ps

        def emit_ctx(kti, pt):
            if kti == 0:
                state["ctx"] = psum.tile(
                    [HD + 1, QC], F32, tag="ctx", bufs=1, name="ctxp"
                )
            first = kti == 0
            last = kti == NST - 1
            for half in range(2):
                sl = slice(half * 512, (half + 1) * 512)
                nc.tensor.matmul(
                    state["ctx"][:, sl], lhsT=vc[kti][:, h, :],
                    rhs=pt[:, sl], start=first, stop=last,
                )

        pts = {}
        for kti in range(NST):
            ps = emit_scores(kti)
            if work_q and work_q[0][0] <= aidx:
                work_q.pop(0)[1]()
            if kti >= LAG:
                emit_ctx(kti - LAG, pts.pop(kti - LAG))
            pt = pt_pool.tile([P, QC], BF16, tag="pt", bufs=12)
            nc.scalar.activation(pt[:], ps[:], Exp, scale=SCALE)
            pts[kti] = pt

        # tail: last LAG ctx tiles + PSUM evacuation + normalization are
        # prepended (in order) to the work-stream, draining during the
        # following attend's first k-tiles
        def tail_ctx(kti):
            def f():
                emit_ctx(kti, pts.pop(kti))
            return f

        def evac():
            csb = small.tile([HD + 1, QC], BF16, tag="csb", bufs=4)
            nc.vector.tensor_copy(csb[:], state["ctx"][:])
            state["csb"] = csb

        head_items = [(0, tail_ctx(kti)) for kti in range(NST - LAG, NST)]
        head_items.append((0, evac))
        head_items.append((0, lambda: normalize(state["csb"], m, r0, qc)))
        work_q[0:0] = head_items

    aidx = 0
    for qc in range(2):
        for h in range(NH):
            attend(aidx, h, qc)
            aidx += 1
    while work_q:
        work_q.pop(0)[1]()
    for st in range(NST // 2, NST):
        outproj_tail(st)


_CACHED = None


def _get_nc():
    global _CACHED
    if _CACHED is None:
        nc = bacc.Bacc(
            "TRN2", target_bir_lowering=False, debug=False, num_devices=8
        )
        hs = nc.dram_tensor("hs", [S, D], BF16, kind="ExternalInput").ap()
        wqkv = nc.dram_tensor("wqkv", [D, 3 * DQ], BF16, kind="ExternalInput").ap()
        wout = nc.dram_tensor("wout", [DQ, D], BF16, kind="ExternalInput").ap()
        out = nc.dram_tensor("out", [S, D], BF16, kind="ExternalOutput").ap()
        build_kernel(nc, out, hs, wqkv, wout)
        nc.compile()
        _CACHED = nc
    return _CACHED


def make_in_maps(hidden_states, w_qkv, w_out):
    in_maps = []
    for c in range(8):
        b, g = divmod(c, 2)
        cols = slice(g * DQ, (g + 1) * DQ)
        wq = w_qkv[:, 0 * D:1 * D][:, cols]
        wk = w_qkv[:, 1 * D:2 * D][:, cols]
        wv = w_qkv[:, 2 * D:3 * D][:, cols]
        bf = ml_dtypes.bfloat16
        in_maps.append({
            "hs": np.ascontiguousarray(hidden_states[b]).astype(bf),
            "wqkv": np.ascontiguousarray(
                np.concatenate([wq, wk, wv], axis=1)
            ).astype(bf),
            "wout": np.ascontiguousarray(
                w_out[g * DQ:(g + 1) * DQ, :]
            ).astype(bf),
        })
    return in_maps


def run(hidden_states, w_qkv, w_out, trace=False):
    nc = _get_nc()
    in_maps = make_in_maps(hidden_states, w_qkv, w_out)
    res = None
    last_err = None
    for _attempt in range(3):
        try:
            res = run_bass_kernel_spmd(
                nc, in_maps, core_ids=list(range(8)), trace=trace
            )
            break
        except Exception as e:  # transient NRT/device hiccups
            last_err = e
    if res is None:
        raise last_err
    out = np.empty((4, S, D), np.float32)
    for b in range(4):
        out[b] = (
            res.results[2 * b]["out"].astype(np.float32)
            + res.results[2 * b + 1]["out"].astype(np.float32)
        )
    return out, res


def kernel(hidden_states, w_qkv, w_out):
    out, _ = run(
        np.asarray(hidden_states), np.asarray(w_qkv), np.asarray(w_out)
    )
    return out
